# revision 52
# baseline (speedup 1.0000x reference)
"""MixHop layer (powers 0,1,2) Trainium2 Bass kernel — v2.

Algorithm (per batch b, one NeuronCore each):
    reference: z_p = lrelu(adj^p @ (x @ W_p)),  out = concat_p z_p
    Key identity: adj @ (x @ W) == (adj @ x) @ W  (adj acts on nodes,
    W on features), so diffuse x ONCE per power level instead of
    diffusing each h_p separately:
        y1 = adj @ x          (768 diffusion cols: (t, f))
        y2 = adj @ y1         (768 diffusion cols)
        z0 = lrelu(x @ W0); z1 = lrelu(y1 @ W1); z2 = lrelu(y2 @ W2)
    That is 2x768 diffusion col-passes vs 1536+768 for the naive
    ordering — a 1.5x cut in tensor-engine work.

Implementation notes:
  - All matmul operands are bf16 (1 PE row/cycle, same as fp32r, but
    half the SBUF/DMA bytes); PSUM accumulation stays fp32. adjT is
    SBUF-resident (8.4 MB bf16), read from HBM exactly once and used
    as the moving operand of BOTH diffusion passes.
  - Diffusion runs in "transposed orientation": out y1T[c, n] with
    c=(t,f) on partitions. lhsT = x[m-block, c-block] (natural x
    layout), rhs = adjT[m-block, all n]. The c-layout pairs two t
    planes per 128 partitions (c = t*64+f), so the per-power weight
    matmul is a single [128,128] block-diagonal (W ⊕ W) stationary
    matmul over each y*T c-block — no transposes for z1/z2/z0.
  - The one real transpose (y1T -> y1 natural, needed as pass-2 lhsT)
    uses the DMA XBAR 16x128 transpose path (bf16 SBUF->SBUF): zero
    tensor-engine cost, ~11 us of otherwise-idle DMA time.
  - Matmul outputs are 512-wide (one PSUM bank). Accumulation chains
    run n-chunk-outer / mb-inner so chunk drains (DVE fp32->bf16 copy)
    overlap the next chunk's 16-matmul chain. Pass-1 cb0 runs mb-outer
    instead so its matmuls chase the 16 streaming adjT slab DMAs.
"""

import os
import sys

if "/opt/trn_rl_repo" not in sys.path:
    sys.path.insert(0, "/opt/trn_rl_repo")

import ml_dtypes
import numpy as np

import concourse.bass as bass
import concourse.tile as tile
from concourse import bacc, mybir
from concourse.bass_utils import run_bass_kernel_spmd

F = 64          # input features
O = 64          # output features per power
N = 2048        # nodes
T = 12          # time steps
NB = N // 128   # 16 node blocks
CB = (T // 2)   # 6 c-blocks (t-pair x 64 features/outputs)
C = T * F       # 768 diffusion columns, c = t*64 + f

F32 = mybir.dt.float32
BF16 = mybir.dt.bfloat16


def build_nc():
    nc = bacc.Bacc("TRN2", target_bir_lowering=False, debug=False, num_devices=8)

    # ---- DRAM I/O ----------------------------------------------------------
    # x2: [(tl f), (th n)] for z0 and (via on-chip XBAR transpose) the
    # natural-layout x used as pass-1 lhsT; t = 2*th + tl
    x2_d = nc.dram_tensor("x2", [128, CB * N], BF16, kind="ExternalInput").ap()
    # xt: [p, (mb c)] natural x tiled on m, c = t*64+f  (pass-1 lhsT slices)
    xt_d = nc.dram_tensor("xt", [128, NB * C], BF16, kind="ExternalInput").ap()
    # adjt[mb, p, n] = adj[n, mb*128+p] — moving operand of both passes
    adjt_d = nc.dram_tensor("adjt", [NB, 128, N], BF16, kind="ExternalInput").ap()
    # wz: [W1blk | W2blk | W0blk], each [128,128] = W ⊕ W over (tl f)x(tl o)
    wz_d = nc.dram_tensor("wz", [128, 384], BF16, kind="ExternalInput").ap()

    # outputs: [(pair tl o), n] fp32, t = 2*pair + tl
    z0_d = nc.dram_tensor("z0", [C, N], F32, kind="ExternalOutput").ap()
    z1_d = nc.dram_tensor("z1", [C, N], F32, kind="ExternalOutput").ap()
    z2_d = nc.dram_tensor("z2", [C, N], F32, kind="ExternalOutput").ap()

    lrelu = mybir.ActivationFunctionType.Lrelu

    with tile.TileContext(nc) as tc:
        with (
            tc.tile_pool(name="wzp", bufs=1) as wzp,
            tc.tile_pool(name="xtp", bufs=1) as xtp,
            tc.tile_pool(name="x2p", bufs=1) as x2p,
            tc.tile_pool(name="adjp", bufs=1) as adjp,
            tc.tile_pool(name="y1p", bufs=1) as y1p,
            tc.tile_pool(name="y1tp", bufs=6) as y1tp,
            tc.tile_pool(name="y2tp", bufs=2) as y2tp,
            tc.tile_pool(name="zstp", bufs=3) as zstp,
            tc.tile_pool(name="acc", bufs=5, space="PSUM") as accp,
            tc.tile_pool(name="zpp", bufs=3, space="PSUM") as zpp,
        ):
            # SP launch order is the head's critical path: wz and xt first,
            # then the adjT slabs with x2 slabs woven in just ahead of the
            # z0 blocks that consume them in cb0's interleaved PE stream.
            wz_t = wzp.tile([128, 384], BF16)
            nc.sync.dma_start(out=wz_t[:], in_=wz_d)
            # xt in three pieces woven into the slab stream: cb0-mb0 only
            # waits for piece 0 (0.77 MB) instead of the whole 3.1 MB load
            xt_sb = xtp.tile([128, NB * C], BF16)
            nc.sync.dma_start(out=xt_sb[:, 0 : 4 * C], in_=xt_d[:, 0 : 4 * C])
            adj_sb = adjp.tile([128, NB * N], BF16)
            x2_sb = x2p.tile([128, CB * N], BF16)
            th_load = 0
            for mb in range(NB):
                if mb < 4:
                    # first slabs in halves: cb0-mb's first two matmuls start
                    # half a slab sooner, right where the PE is coldest
                    nc.sync.dma_start(
                        out=adj_sb[:, mb * N : mb * N + 1024],
                        in_=adjt_d[mb][:, 0:1024],
                    )
                    nc.sync.dma_start(
                        out=adj_sb[:, mb * N + 1024 : (mb + 1) * N],
                        in_=adjt_d[mb][:, 1024:2048],
                    )
                else:
                    nc.sync.dma_start(
                        out=adj_sb[:, mb * N : (mb + 1) * N], in_=adjt_d[mb]
                    )
                if mb == 1:
                    nc.sync.dma_start(
                        out=xt_sb[:, 4 * C : 10 * C], in_=xt_d[:, 4 * C : 10 * C]
                    )
                if mb == 4:
                    nc.sync.dma_start(
                        out=xt_sb[:, 10 * C : NB * C], in_=xt_d[:, 10 * C : NB * C]
                    )
                if mb % 3 == 1 and th_load < CB:
                    nc.sync.dma_start(
                        out=x2_sb[:, th_load * N : (th_load + 1) * N],
                        in_=x2_d[:, th_load * N : (th_load + 1) * N],
                    )
                    th_load += 1
            while th_load < CB:
                nc.sync.dma_start(
                    out=x2_sb[:, th_load * N : (th_load + 1) * N],
                    in_=x2_d[:, th_load * N : (th_load + 1) * N],
                )
                th_load += 1
            y1_sb = y1p.tile([128, NB * C], BF16)

            def z_block(dst_d, widx, rhs_sb, col0, row0, chunk_stores=False):
                """One [128, N] output block: blockdiag W matmul + lrelu + store.

                Stores ride the Activation HWDGE queue (SP carries the XBAR
                transposes). chunk_stores=True (used for the final block only)
                issues per-512 stores behind each lrelu to shorten the tail.
                """
                zst = zstp.tile([128, N], F32, tag="zst")
                for s in range(4):
                    zp = zpp.tile([128, 512], F32, tag="zp")
                    nc.tensor.matmul(
                        zp[:],
                        wz_t[:, widx * 128 : (widx + 1) * 128],
                        rhs_sb[:, col0 + s * 512 : col0 + (s + 1) * 512],
                        start=True,
                        stop=True,
                    )
                    nc.scalar.activation(
                        zst[:, s * 512 : (s + 1) * 512], zp[:], lrelu, alpha=0.01
                    )
                    if chunk_stores:
                        nc.scalar.dma_start(
                            out=dst_d[row0 : row0 + 128, s * 512 : (s + 1) * 512],
                            in_=zst[:, s * 512 : (s + 1) * 512],
                        )
                if not chunk_stores:
                    nc.scalar.dma_start(out=dst_d[row0 : row0 + 128, :], in_=zst[:])

            # ---- pass 1: y1T[c, n] = sum_m x[m, c] adj[n, m] ---------------
            # cb0 runs mb-outer first thing, chasing the adjT slab stream;
            # the z0 blocks follow it, paced by the trailing x2 loads.
            for cb in range(CB):
                y1t = y1tp.tile([128, N], BF16, tag="y1t")
                if cb == 0:
                    accs = [
                        accp.tile([128, 512], F32, tag="acc", name=f"acc0_{s}")
                        for s in range(4)
                    ]
                    th_z0 = 0
                    for mb in range(NB):
                        lhsT = xt_sb[:, mb * C + cb * 128 : mb * C + (cb + 1) * 128]
                        for s in range(4):
                            nc.tensor.matmul(
                                accs[s][:],
                                lhsT,
                                adj_sb[:, mb * N + s * 512 : mb * N + (s + 1) * 512],
                                start=(mb == 0),
                                stop=(mb == NB - 1),
                            )
                        # z0 = lrelu(x @ W0) blocks fill the slab-wait gaps
                        if mb % 3 == 2 and th_z0 < CB:
                            z_block(z0_d, 2, x2_sb, th_z0 * N, th_z0 * 128)
                            th_z0 += 1
                    while th_z0 < CB:
                        z_block(z0_d, 2, x2_sb, th_z0 * N, th_z0 * 128)
                        th_z0 += 1
                    for s in range(4):
                        nc.vector.tensor_copy(
                            y1t[:, s * 512 : (s + 1) * 512], accs[s][:]
                        )
                    z_block(z1_d, 0, y1t, 0, 0)
                else:
                    # chunk-outer with the z1 matmul + lrelu fused per chunk:
                    # each chunk's weight matmul fires as soon as its CAST
                    # lands instead of after all four chains, shortening the
                    # dependency tail of every cb.
                    zst = zstp.tile([128, N], F32, tag="zst")
                    for s in range(4):
                        acc = accp.tile([128, 512], F32, tag="acc")
                        for mb in range(NB):
                            nc.tensor.matmul(
                                acc[:],
                                xt_sb[:, mb * C + cb * 128 : mb * C + (cb + 1) * 128],
                                adj_sb[:, mb * N + s * 512 : mb * N + (s + 1) * 512],
                                start=(mb == 0),
                                stop=(mb == NB - 1),
                            )
                        nc.vector.tensor_copy(y1t[:, s * 512 : (s + 1) * 512], acc[:])
                        zp = zpp.tile([128, 512], F32, tag="zp")
                        nc.tensor.matmul(
                            zp[:],
                            wz_t[:, 0:128],
                            y1t[:, s * 512 : (s + 1) * 512],
                            start=True,
                            stop=True,
                        )
                        nc.scalar.activation(
                            zst[:, s * 512 : (s + 1) * 512], zp[:], lrelu, alpha=0.01
                        )
                    nc.scalar.dma_start(
                        out=z1_d[cb * 128 : (cb + 1) * 128, :], in_=zst[:]
                    )
                # y1T -> y1 natural via ONE DMA XBAR transpose per cb: the
                # 3D out AP scatters block mb to y1_sb[:, mb*C + cb*128]
                # (sim semantics: out[a, b, c] = in[c, b*128 + a])
                nc.sync.dma_start(
                    out=y1_sb[:]
                    .rearrange("p (mb c) -> p mb c", mb=NB)[
                        :, :, cb * 128 : (cb + 1) * 128
                    ],
                    in_=y1t[:],
                    transpose=True,
                )

            # ---- pass 2: y2T[c, n] = sum_m y1[m, c] adj[n, m] --------------
            # Same fused per-chunk z2 pipeline; the final cb stores per
            # chunk so the kernel tail is one chunk deep, not four.
            for cb in range(CB):
                y2t = y2tp.tile([128, N], BF16, tag="y2t")
                zst = zstp.tile([128, N], F32, tag="zst")
                last = cb == CB - 1
                for s in range(4):
                    acc = accp.tile([128, 512], F32, tag="acc")
                    for mb in range(NB):
                        nc.tensor.matmul(
                            acc[:],
                            y1_sb[:, mb * C + cb * 128 : mb * C + (cb + 1) * 128],
                            adj_sb[:, mb * N + s * 512 : mb * N + (s + 1) * 512],
                            start=(mb == 0),
                            stop=(mb == NB - 1),
                        )
                    nc.vector.tensor_copy(y2t[:, s * 512 : (s + 1) * 512], acc[:])
                    zp = zpp.tile([128, 512], F32, tag="zp")
                    nc.tensor.matmul(
                        zp[:],
                        wz_t[:, 128:256],
                        y2t[:, s * 512 : (s + 1) * 512],
                        start=True,
                        stop=True,
                    )
                    nc.scalar.activation(
                        zst[:, s * 512 : (s + 1) * 512], zp[:], lrelu, alpha=0.01
                    )
                    if last:
                        nc.scalar.dma_start(
                            out=z2_d[cb * 128 : (cb + 1) * 128, s * 512 : (s + 1) * 512],
                            in_=zst[:, s * 512 : (s + 1) * 512],
                        )
                if not last:
                    nc.scalar.dma_start(
                        out=z2_d[cb * 128 : (cb + 1) * 128, :], in_=zst[:]
                    )

    nc.finalize()
    return nc


_NC = None
LAST_RESULTS = None  # stashed BassKernelResults for test harnesses


def kernel(x, adj, W0, b0, W1, b1, W2, b2):
    """Full inputs in, full output out. Shards batch b -> core b."""
    global _NC, LAST_RESULTS
    x = np.asarray(x, dtype=np.float32)
    adj = np.asarray(adj, dtype=np.float32)
    W0 = np.asarray(W0, dtype=np.float32)
    W1 = np.asarray(W1, dtype=np.float32)
    W2 = np.asarray(W2, dtype=np.float32)
    B = x.shape[0]
    assert B == 8 and x.shape == (8, F, N, T) and adj.shape == (8, N, N)

    if _NC is None:
        _NC = build_nc()

    bf = ml_dtypes.bfloat16
    # x2[b, tl*64+f, th*2048+n] = x[b, f, n, 2*th+tl]
    x2 = (
        np.ascontiguousarray(
            x.transpose(0, 3, 1, 2)
            .reshape(B, CB, 2, F, N)
            .transpose(0, 2, 3, 1, 4)
        )
        .reshape(B, 128, CB * N)
        .astype(bf)
    )
    # xt[b, ml, mb*768 + t*64+f] = x[b, f, mb*128+ml, t]
    xt = (
        np.ascontiguousarray(
            x.transpose(0, 2, 3, 1).reshape(B, NB, 128, C).transpose(0, 2, 1, 3)
        )
        .reshape(B, 128, NB * C)
        .astype(bf)
    )
    # adjt[b, mb, p, n] = adj[b, n, mb*128+p]
    adjt = (
        np.ascontiguousarray(adj.transpose(0, 2, 1)).reshape(B, NB, 128, N).astype(bf)
    )
    wz = np.zeros((128, 384), dtype=np.float32)
    for tl in range(2):
        r = slice(tl * 64, tl * 64 + 64)
        wz[r, tl * 64 : tl * 64 + 64] = W1
        wz[r, 128 + tl * 64 : 128 + tl * 64 + 64] = W2
        wz[r, 256 + tl * 64 : 256 + tl * 64 + 64] = W0
    wz = wz.astype(bf)

    in_maps = [
        {"x2": x2[b], "xt": xt[b], "adjt": adjt[b], "wz": wz} for b in range(B)
    ]
    nwarm = int(os.environ.get("KERNEL_WARMUP_RUNS", "0"))
    for _ in range(nwarm):
        run_bass_kernel_spmd(_NC, in_maps, core_ids=list(range(8)))
    res = run_bass_kernel_spmd(_NC, in_maps, core_ids=list(range(8)))
    LAST_RESULTS = res

    out = np.empty((B, 3 * O, N, T), dtype=np.float32)
    for b in range(B):
        r = res.results[b]
        for i, key in enumerate(("z0", "z1", "z2")):
            # [(pair tl o), n] -> [o, n, t] with t = 2*pair + tl
            z = r[key].reshape(CB, 2, O, N).transpose(2, 3, 0, 1).reshape(O, N, T)
            out[b, i * O : (i + 1) * O] = z
    del b0, b1, b2
    return out
